# revision 7
# baseline (speedup 1.0000x reference)
"""Damped EMA (first-order IIR) as a short FIR convolution on Trainium2.

h[t] = alpha*x[t] + (1-alpha)*h[t-1]  ==  h = conv(x, w), w[tau] = alpha*r^tau,
r = 1-alpha.  For the problem's alpha (0.9) the kernel decays below the bf16
wire-format quantum within 9 taps, so a truncated FIR is exact to ~1e-8
relative on top of the ~2.4e-3 bf16 I/O quantization (gate is 2e-2).

Sharding: 8 cores = batch (4) x T-halves (2); each core owns a contiguous
(2048, 1024) output block.  No inter-core communication.

Per core (raw Bass, manual semaphores):
  * the host packs the shard into 18 OVERLAPPING 128-row tiles (120 new
    rows + 8-row causal halo baked into each tile, zero/neighbor padded),
    partition-major in DRAM so every load line is 4-8KB contiguous;
  * one [128,120] banded-Toeplitz weight matrix turns each tile into 120
    output rows with a SINGLE TensorE matmul per 512-col group (36 total);
    bf16 operands use the PE's fast path (fp16 measured ~2x slower);
  * Sync HWDGE ring carries the weights + first load group THEN the
    stores (so stores stream as soon as copies land); Scalar HWDGE ring
    carries the remaining load groups (after its auto-inserted
    ACT_TABLE_LOAD, which no longer delays the critical first tiles);
  * PSUM->SBUF bf16 downcast copies split between VectorE (g=0) and
    ScalarE (g=1); paired stores (2 chunks per DMA via 3D APs) keep the
    sync sequencer light; host upcasts the bf16 output to fp32.

Wire traffic is 8.95 MB/core vs a ~410 B/ns per-core HBM budget, plus
~6.5us fixed framework preamble, ~1.7us DMA ring spin-up, and ~2us of
counted epilogue.
"""

import sys

import numpy as np

if "/opt/trn_rl_repo" not in sys.path:
    sys.path.insert(0, "/opt/trn_rl_repo")

B, T, D = 4, 4096, 1024
N_CORES = 8
TG = T // 2  # output rows per core (batch x T-half sharding)
C = 120  # output rows per full chunk
HALO = 8  # causal halo rows per tile (supports n_taps <= 9)
NT = 18  # tiles per core: 17 full chunks (2040 rows) + 1 tail chunk (8 rows)
TAIL = TG - 17 * C  # 8
GROUPS = [(0, 2), (2, 6), (6, 10), (10, 14), (14, 18)]  # tile ranges per load DMA

LAST_EXEC_TIME_NS = None
LAST_TRACE_PATH = None

_NC_CACHE = {}


def _rows(c: int) -> int:
    return C if c < NT - 1 else TAIL


def _n_taps(a: float, r: float) -> int:
    """Taps to keep so the dropped tail is <= ~1e-8 relative."""
    if a == 0.0 or abs(r) == 0.0:
        return 1
    ar = abs(r)
    assert ar < 1.0, f"unstable EMA (|1-alpha|={ar} >= 1), cannot truncate"
    return max(1, int(np.ceil(-8.0 / np.log10(ar))))


def _build_program():
    import concourse.bacc as bacc
    import concourse.mybir as mybir

    bf16 = mybir.dt.bfloat16

    nc = bacc.Bacc(
        "TRN2",
        target_bir_lowering=False,
        debug=False,
        num_devices=N_CORES,
        dynamic_dma_scratch_size=49152,
    )
    xd = nc.dram_tensor("x", [128, NT * D], bf16, kind="ExternalInput").ap()
    wd = nc.dram_tensor("w", [128, 128], bf16, kind="ExternalInput").ap()
    od = nc.dram_tensor("out", [TG, D], bf16, kind="ExternalOutput").ap()

    xs = nc.alloc_sbuf_tensor("xs", [128, NT * D], bf16).ap()
    os_ = nc.alloc_sbuf_tensor("os", [128, NT * D], bf16).ap()
    wt = nc.alloc_sbuf_tensor("wt", [128, 128], bf16).ap()
    ps = [
        nc.alloc_psum_tensor(f"ps{b}", [128, 512], mybir.dt.float32).ap()
        for b in range(8)
    ]

    def group_of_tile(n):
        for gi, (a, b) in enumerate(GROUPS):
            if a <= n < b:
                return gi
        raise ValueError(n)

    with (
        nc.Block(no_gpsimd_drain=True) as block,
        nc.semaphore("s_w") as s_w,
        nc.semaphore("s_ld0") as s_ld0,
        nc.semaphore("s_ld") as s_ld,
        nc.semaphore("s_mm") as s_mm,
        nc.semaphore("s_cv") as s_cv,
        nc.semaphore("s_cs") as s_cs,
        nc.semaphore("s_st") as s_st,
    ):

        @block.tensor
        def _(te):
            te.wait_ge(s_w, 16)
            last_g = 0
            for u in range(2 * NT):
                c, g = divmod(u, 2)
                need_g = group_of_tile(c)
                if need_g == 0:
                    if last_g == 0 and u == 0:
                        te.wait_ge(s_ld0, 16)
                elif need_g > last_g:
                    te.wait_ge(s_ld, 16 * need_g)
                    last_g = need_g
                if u >= 8:
                    # PSUM bank WAR: wait for the copy that drained this bank
                    cc, gg = divmod(u - 8, 2)
                    te.wait_ge(s_cv if gg == 0 else s_cs, cc + 1)
                r = _rows(c)
                te.matmul(
                    ps[u % 8][0:r, :],
                    wt[:, 0:r],
                    xs[:, c * D + g * 512 : c * D + g * 512 + 512],
                    start=True,
                    stop=True,
                ).then_inc(s_mm, 1)

        @block.vector
        def _(ve):
            for c in range(NT):
                u = 2 * c
                r = _rows(c)
                ve.wait_ge(s_mm, u + 1)
                ve.tensor_copy(
                    os_[0:r, c * D : c * D + 512], ps[u % 8][0:r, :]
                ).then_inc(s_cv, 1)

        @block.scalar
        def _(se):
            for a, b in GROUPS[1:]:
                se.dma_start(out=xs[:, a * D : b * D], in_=xd[:, a * D : b * D]).then_inc(
                    s_ld, 16
                )
            for c in range(NT):
                u = 2 * c + 1
                r = _rows(c)
                se.wait_ge(s_mm, u + 1)
                se.copy(
                    os_[0:r, c * D + 512 : (c + 1) * D], ps[u % 8][0:r, :]
                ).then_inc(s_cs, 1)

        @block.sync
        def _(sy):
            sy.dma_start(out=wt[:, :], in_=wd[:, :]).then_inc(s_w, 16)
            a, b = GROUPS[0]
            sy.dma_start(out=xs[:, a * D : b * D], in_=xd[:, a * D : b * D]).then_inc(
                s_ld0, 16
            )
            n_st = 0
            # paired stores (2 chunks per DMA) keep the sync sequencer light
            for c in range(0, NT - 2, 2):
                sy.wait_ge(s_cv, c + 2)
                sy.wait_ge(s_cs, c + 2)
                out2 = od[c * C : (c + 2) * C, :].rearrange("(j p) d -> p j d", p=C)
                in2 = os_[0:C, c * D : (c + 2) * D].rearrange("p (j d) -> p j d", d=D)
                sy.dma_start(out=out2, in_=in2).then_inc(s_st, 16)
                n_st += 1
            for c in (NT - 2, NT - 1):
                r = _rows(c)
                sy.wait_ge(s_cv, c + 1)
                sy.wait_ge(s_cs, c + 1)
                sy.dma_start(
                    out=od[c * C : c * C + r, :], in_=os_[0:r, c * D : (c + 1) * D]
                ).then_inc(s_st, 16)
                n_st += 1
            sy.wait_ge(s_st, 16 * n_st)

    nc.compile()
    return nc


def kernel(x: np.ndarray, alpha: np.ndarray) -> np.ndarray:
    global LAST_EXEC_TIME_NS, LAST_TRACE_PATH
    import ml_dtypes
    from concourse.bass_utils import run_bass_kernel_spmd

    bf = ml_dtypes.bfloat16
    x = np.ascontiguousarray(np.asarray(x, dtype=np.float32))
    assert x.shape == (B, T, D), x.shape
    a = float(np.asarray(alpha, dtype=np.float32).reshape(-1)[0])
    r = np.float32(1.0) - np.float32(a)

    n_taps = _n_taps(a, float(r))
    if n_taps > HALO + 1:
        # EMA memory longer than the baked-in halo — out of scope for the
        # tuned TRN path; exact host-side scan keeps the answer right.
        h = np.empty_like(x)
        carry = np.zeros((B, D), dtype=np.float32)
        for t in range(T):
            carry = a * x[:, t, :] + (1.0 - a) * carry
            h[:, t, :] = carry
        return h

    # FIR taps, fp32 like the reference
    powers = np.arange(n_taps, dtype=np.float32)
    w = (np.float32(a) * np.power(r, powers, dtype=np.float32)).astype(np.float32)

    # Banded Toeplitz: Wb[k, m] = w[m + HALO - k], nonzero band fully inside
    # the 128-row tile for all m in [0, C)
    kk = np.arange(128)[:, None]
    mm = np.arange(128)[None, :]
    Wb = np.zeros((128, 128), dtype=np.float32)
    tap = mm + HALO - kk
    v = (tap >= 0) & (tap < n_taps) & (mm < C)
    Wb[v] = w[tap[v]]
    Wb16 = Wb.astype(bf)

    nc = _NC_CACHE.get("prog")
    if nc is None:
        nc = _build_program()
        _NC_CACHE["prog"] = nc

    PAD = (NT - 1) * C + 128 - HALO - TG  # rows of zero padding after the shard
    in_maps = []
    for core in range(N_CORES):
        b, half = divmod(core, 2)
        base = half * TG
        if half == 0:
            halo = np.zeros((HALO, D), dtype=np.float32)
        else:
            halo = x[b, base - HALO : base, :]
        # P[j] = shard row j - HALO (halo rows first, zero tail after)
        P = np.concatenate(
            [halo, x[b, base : base + TG, :], np.zeros((PAD, D), dtype=np.float32)],
            axis=0,
        ).astype(bf)
        s0, s1 = P.strides
        tiles = np.lib.stride_tricks.as_strided(P, (NT, 128, D), (C * s0, s0, s1))
        xp = np.ascontiguousarray(tiles.transpose(1, 0, 2).reshape(128, NT * D))
        in_maps.append({"x": xp, "w": Wb16})

    res = run_bass_kernel_spmd(nc, in_maps, list(range(N_CORES)))
    LAST_EXEC_TIME_NS = res.exec_time_ns
    it = res.instructions_and_trace
    LAST_TRACE_PATH = it[1] if it else None

    h = np.empty((B, T, D), dtype=np.float32)
    for core in range(N_CORES):
        b, half = divmod(core, 2)
        base = half * TG
        h[b, base : base + TG, :] = res.results[core]["out"].astype(np.float32)
    return h
